# revision 6
# baseline (speedup 1.0000x reference)
"""D4-pool Trainium2 kernel.

x: [256, 128, 64, 64] f32. Groups of 8 consecutive batch entries hold the 8
D4 orientations of one image; undo each orientation and mean over the group,
giving [32, 128, 64, 64].

Sharding: data-parallel over the group dim — core k gets groups [4k, 4k+4)
(batch entries [32k, 32k+32)), so the reduce is fully device-local.

Layout trick: with C (=128) on SBUF partitions and (H, W) on the free dim,
every D4 inverse transform is pure free-dim address arithmetic (stride ±1 /
±64 access patterns) — no transpose instructions, no partition movement.
Per partition, the required inverse-transform reads are:
  o=0: A[h, w]          o=1: A[w, 63-h]     o=2: A[63-h, 63-w]
  o=3: A[63-w, h]       o=4: A[h, 63-w]     o=5: A[w, h]
  o=6: A[63-h, w]       o=7: A[63-w, 63-h]

Two accumulators per group so only ONE DVE op per group pays the
slow inner-stride-64 (transposed) read:
  acc  [c,h,w]: init = x0*1/8 (ACT), += o=2,4,6 (flip APs, stride ±1)
  accT [c,w,h]: init = x5*1/8 (ACT; pure transpose == contiguous),
                += o=1,3,7 (flips in transposed coords, stride ±1)
The 1/8 scale folds into every accumulate (DVE STT: term*s + acc).

Tail scheduling: the DMA engines are the bottleneck (~27 GB/s x 16 =
~430 GB/s line rate); any compute serialized after the last load is pure
loss. So one "early" group E has its first 7 orientations loaded and fully
reduced during the pipeline-fill phase (DVE is idle there), its accumulator
stays resident, and the kernel ends with E's o=2 tile streamed in four
quarter-tiles: load -> STT -> store, pipelined at ~1.2 us granularity so the
DMA engines never drain while waiting on compute.
"""

import sys

for _p in ("/opt/trn_rl_repo",):
    if _p not in sys.path:
        sys.path.insert(0, _p)

import numpy as np

import concourse.bacc as bacc
import concourse.mybir as mybir
from concourse.bass_utils import run_bass_kernel_spmd
from concourse.tile import TileContext

N_CORES = 8
B, C, H, W = 256, 128, 64, 64
ENTRIES_PER_CORE = B // N_CORES          # 32 batch entries
GROUPS_PER_CORE = ENTRIES_PER_CORE // 8  # 4 groups of 8 orientations

# accT-side APs: accT[c,a,b] += x1[c,a,63-b] / x3[c,63-a,b] / x7[c,63-a,63-b]
_ACCT_AP = {
    1: lambda t: t[:, :, ::-1],
    3: lambda t: t[:, ::-1, :],
    7: lambda t: t[:, ::-1, ::-1],
}
# acc-side APs: acc[c,h,w] += x2[c,63-h,63-w] / x4[c,h,63-w] / x6[c,63-h,w]
_ACC_AP = {
    2: lambda t: t[:, ::-1, ::-1],
    4: lambda t: t[:, :, ::-1],
    6: lambda t: t[:, ::-1, :],
}


def build_nc(groups: int = GROUPS_PER_CORE) -> bacc.Bacc:
    f32 = mybir.dt.float32
    bf16 = mybir.dt.bfloat16
    mult, add = mybir.AluOpType.mult, mybir.AluOpType.add
    nc = bacc.Bacc()
    x = nc.declare_dram_parameter("x", [groups * 8, C, H, W], f32, isOutput=False)
    # Stores go out in bf16 (rel err ~1e-3 << the 2e-2 gate); the f32
    # accumulate happens on-chip, only the final store payload is halved.
    y = nc.declare_dram_parameter("y", [groups, C, H, W], bf16, isOutput=True)

    E = groups - 1           # the "early" group, finished last in quarters
    HH = H // 2              # half-tile rows
    QH = H // 4              # quarter-tile rows for the tail pipeline

    with TileContext(nc) as tc, nc.allow_low_precision("bf16 store of mean"):
        with (
            tc.tile_pool(name="xin", bufs=6) as xin_pool,
            tc.tile_pool(name="acc", bufs=2) as acc_pool,
            tc.tile_pool(name="accT", bufs=2) as accT_pool,
            tc.tile_pool(name="accE", bufs=1) as accE_pool,
            tc.tile_pool(name="yb", bufs=2) as yb_pool,
        ):
            def load(b):
                xt = xin_pool.tile([C, H, W], f32, tag="xin")
                nc.sync.dma_start(xt[:, :, :], x[b])
                return xt

            def acct_phase(g, accT):
                """Loads + raw-sum of the accT side: o=5 (init), 1, 3, 7."""
                xt = load(8 * g + 5)
                nc.scalar.copy(accT[:, :, :], xt[:, :, :])
                for o in (1, 3, 7):
                    xt = load(8 * g + o)
                    nc.vector.tensor_add(
                        accT[:, :, :], accT[:, :, :], _ACCT_AP[o](xt),
                    )

            def combine(acc, accT):
                for h0 in (0, HH):
                    hs = slice(h0, h0 + HH)
                    nc.vector.tensor_add(
                        acc[:, hs, :], acc[:, hs, :],
                        accT[:, :, hs].transpose([0, 2, 1]),
                    )

            def acc_add(acc, g, o):
                """acc += o-term: DVE takes rows [0,32), Pool rows [32,64).

                Accumulation is a raw sum (the 1/8 is folded into the final
                ACT scale-copy), so these are plain tensor-tensor adds which
                the Pool engine's ISA supports. The halves are write-disjoint
                so the two engines run concurrently; this keeps DVE's
                per-group total under the per-group DMA time even in the
                chip's slow-clock state.
                """
                xt = load(8 * g + o)
                lo, hi = slice(0, HH), slice(HH, H)
                nc.vector.tensor_add(
                    acc[:, lo, :], acc[:, lo, :], _ACC_AP[o](xt)[:, lo, :],
                )
                nc.gpsimd.tensor_add(
                    acc[:, hi, :], acc[:, hi, :], _ACC_AP[o](xt)[:, hi, :],
                )

            # ---- early group E: everything except o=2, during pipeline fill
            accE = accE_pool.tile([C, H, W], f32, tag="accE")
            xt = load(8 * E + 0)
            nc.scalar.copy(accE[:, :, :], xt[:, :, :])
            accTE = accT_pool.tile([C, H, W], f32, tag="accT")
            acct_phase(E, accTE)
            combine(accE, accTE)
            acc_add(accE, E, 4)
            acc_add(accE, E, 6)

            # ---- steady-state groups
            for g in range(groups - 1):
                acc = acc_pool.tile([C, H, W], f32, tag="acc")
                xt = load(8 * g + 0)
                nc.scalar.copy(acc[:, :, :], xt[:, :, :])
                accT = accT_pool.tile([C, H, W], f32, tag="accT")
                acct_phase(g, accT)
                combine(acc, accT)
                acc_add(acc, g, 4)
                acc_add(acc, g, 6)
                xt2 = load(8 * g + 2)
                for h0 in (0, HH):
                    hs = slice(h0, h0 + HH)
                    nc.vector.tensor_add(
                        acc[:, hs, :], acc[:, hs, :],
                        _ACC_AP[2](xt2)[:, hs, :],
                    )
                    ybh = yb_pool.tile([C, HH, W], bf16, tag="ybh")
                    # Final 1/8 scale + bf16 downcast on the ACT engine.
                    nc.scalar.mul(ybh[:, :, :], acc[:, hs, :], 0.125)
                    # Store on the ACT HWDGE queue — keeps the compute-gated
                    # store from head-of-line blocking loads on sync's queue.
                    nc.scalar.dma_start(y[g][:, hs, :], ybh[:, :, :])

            # ---- tail: E's o=2 in quarter-tiles, load->add->scale->store
            for q in range(4):
                hq = slice(QH * q, QH * (q + 1))
                # output rows [16q,16q+16) read source rows [48-16q, 64-16q)
                src = slice(H - QH * (q + 1), H - QH * q)
                xq = xin_pool.tile([C, QH, W], f32, tag="xq", bufs=4)
                nc.sync.dma_start(xq[:, :, :], x[8 * E + 2][:, src, :])
                nc.vector.tensor_add(
                    accE[:, hq, :], accE[:, hq, :], xq[:, ::-1, ::-1],
                )
                ybq = yb_pool.tile([C, QH, W], bf16, tag="ybq", bufs=3)
                nc.scalar.mul(ybq[:, :, :], accE[:, hq, :], 0.125)
                nc.scalar.dma_start(y[E][:, hq, :], ybq[:, :, :])
    nc.compile()
    return nc


_NC_CACHE: list = []


def run(x: np.ndarray, trace: bool = False, **spmd_kwargs):
    """Shard, run on all 8 cores, gather. Returns (output, BassKernelResults)."""
    x = np.ascontiguousarray(x, dtype=np.float32)
    assert x.shape == (B, C, H, W), x.shape
    shards = x.reshape(N_CORES, ENTRIES_PER_CORE, C, H, W)
    if not _NC_CACHE:
        _NC_CACHE.append(build_nc())
    nc = _NC_CACHE[0]
    in_maps = [{"x": shards[i]} for i in range(N_CORES)]
    res = run_bass_kernel_spmd(
        nc, in_maps, list(range(N_CORES)), trace=trace, **spmd_kwargs
    )
    out = np.concatenate(
        [np.asarray(res.results[i]["y"]).astype(np.float32) for i in range(N_CORES)],
        axis=0,
    )
    return out, res


def kernel(x: np.ndarray) -> np.ndarray:
    out, _ = run(x)
    return out


# revision 7
# speedup vs baseline: 1.0189x; 1.0189x over previous
"""D4-pool Trainium2 kernel.

x: [256, 128, 64, 64] f32. Groups of 8 consecutive batch entries hold the 8
D4 orientations of one image; undo each orientation and mean over the group,
giving [32, 128, 64, 64].

Sharding: data-parallel over the group dim — core k gets groups [4k, 4k+4)
(batch entries [32k, 32k+32)), so the reduce is fully device-local.

Layout trick: with C (=128) on SBUF partitions and (H, W) on the free dim,
every D4 inverse transform is pure free-dim address arithmetic (stride ±1 /
±64 access patterns) — no transpose instructions, no partition movement.
Per partition, the required inverse-transform reads are:
  o=0: A[h, w]          o=1: A[w, 63-h]     o=2: A[63-h, 63-w]
  o=3: A[63-w, h]       o=4: A[h, 63-w]     o=5: A[w, h]
  o=6: A[63-h, w]       o=7: A[63-w, 63-h]

Two accumulators per group so only ONE DVE op per group pays the
slow inner-stride-64 (transposed) read:
  acc  [c,h,w]: init = x0*1/8 (ACT), += o=2,4,6 (flip APs, stride ±1)
  accT [c,w,h]: init = x5*1/8 (ACT; pure transpose == contiguous),
                += o=1,3,7 (flips in transposed coords, stride ±1)
The 1/8 scale folds into every accumulate (DVE STT: term*s + acc).

Scheduling: the 16 DMA engines are the bottleneck (~27 GB/s each, ~430 GB/s
aggregate); any compute serialized after the last load is pure loss, so the
load order is phase-split globally:
  phase A: every group's o=0 (acc init) + transpose-side o=5,1,3,7 —
           all four combines (the only transposed-read DVE ops) execute
           mid-stream, overlapped with phase-B loads;
  phase B: acc-side o=4,6,2 per group; the o=2 term is fused with the bf16
           downcast (out = x2*1/8 + acc, bf16 out) and stored immediately;
           the final group's o=2 streams in four quarter-tiles so the
           load->STT->store chain after the last load is ~2 us.
Stores go out in bf16 (rel err ~1.7e-3, gate is 2e-2), halving store
traffic; accumulation stays f32 on-chip.
"""

import sys

for _p in ("/opt/trn_rl_repo",):
    if _p not in sys.path:
        sys.path.insert(0, _p)

import numpy as np

import concourse.bacc as bacc
import concourse.mybir as mybir
from concourse.bass_utils import run_bass_kernel_spmd
from concourse.tile import TileContext

N_CORES = 8
B, C, H, W = 256, 128, 64, 64
ENTRIES_PER_CORE = B // N_CORES          # 32 batch entries
GROUPS_PER_CORE = ENTRIES_PER_CORE // 8  # 4 groups of 8 orientations

# accT-side APs: accT[c,a,b] += x1[c,a,63-b] / x3[c,63-a,b] / x7[c,63-a,63-b]
_ACCT_AP = {
    1: lambda t: t[:, :, ::-1],
    3: lambda t: t[:, ::-1, :],
    7: lambda t: t[:, ::-1, ::-1],
}
# acc-side APs: acc[c,h,w] += x2[c,63-h,63-w] / x4[c,h,63-w] / x6[c,63-h,w]
_ACC_AP = {
    2: lambda t: t[:, ::-1, ::-1],
    4: lambda t: t[:, :, ::-1],
    6: lambda t: t[:, ::-1, :],
}


def build_nc(groups: int = GROUPS_PER_CORE) -> bacc.Bacc:
    f32 = mybir.dt.float32
    bf16 = mybir.dt.bfloat16
    mult, add = mybir.AluOpType.mult, mybir.AluOpType.add
    nc = bacc.Bacc()
    x = nc.declare_dram_parameter("x", [groups * 8, C, H, W], f32, isOutput=False)
    y = nc.declare_dram_parameter("y", [groups, C, H, W], bf16, isOutput=True)

    L = groups - 1           # last group: o=2 streamed as quarter-tiles
    HH = H // 2              # half-tile rows
    QH = H // 4              # quarter-tile rows for the tail pipeline

    with TileContext(nc) as tc, nc.allow_low_precision("bf16 store of mean"):
        with (
            tc.tile_pool(name="xin", bufs=5) as xin_pool,
            tc.tile_pool(name="acc", bufs=groups) as acc_pool,
            tc.tile_pool(name="accT", bufs=2) as accT_pool,
            tc.tile_pool(name="yb", bufs=2) as yb_pool,
        ):
            def load(b):
                xt = xin_pool.tile([C, H, W], f32, tag="xin")
                nc.sync.dma_start(xt[:, :, :], x[b])
                return xt

            def stt(dst, src_ap, hs=slice(None)):
                nc.vector.scalar_tensor_tensor(
                    dst[:, hs, :], src_ap, 0.125, dst[:, hs, :], mult, add,
                )

            accs = []
            # ---- phase A: acc inits + transpose side + combines
            for g in range(groups):
                xt = load(8 * g + 0)
                acc = acc_pool.tile([C, H, W], f32, tag="acc")
                nc.scalar.mul(acc[:, :, :], xt[:, :, :], 0.125)
                accs.append(acc)
                xt = load(8 * g + 5)
                accT = accT_pool.tile([C, H, W], f32, tag="accT")
                nc.scalar.mul(accT[:, :, :], xt[:, :, :], 0.125)
                for o in (1, 3, 7):
                    xt = load(8 * g + o)
                    stt(accT, _ACCT_AP[o](xt))
                # combine: the one transposed-read op per group, mid-stream
                for h0 in (0, HH):
                    hs = slice(h0, h0 + HH)
                    nc.vector.tensor_add(
                        acc[:, hs, :], acc[:, hs, :],
                        accT[:, :, hs].transpose([0, 2, 1]),
                    )

            # ---- phase B: acc side + fused downcast + stores
            for g in range(groups):
                acc = accs[g]
                for o in (4, 6):
                    xt = load(8 * g + o)
                    stt(acc, _ACC_AP[o](xt))
                if g != L:
                    xt2 = load(8 * g + 2)
                    for h0 in (0, HH):
                        hs = slice(h0, h0 + HH)
                        ybh = yb_pool.tile([C, HH, W], bf16, tag="ybh")
                        nc.vector.scalar_tensor_tensor(
                            ybh[:, :, :], _ACC_AP[2](xt2)[:, hs, :], 0.125,
                            acc[:, hs, :], mult, add,
                        )
                        # Store on the ACT HWDGE queue — keeps the
                        # compute-gated store from head-of-line blocking
                        # loads on sync's queue.
                        nc.scalar.dma_start(y[g][:, hs, :], ybh[:, :, :])

            # ---- tail: last group's o=2 in quarters, load->STT->store
            for q in range(4):
                hq = slice(QH * q, QH * (q + 1))
                # output rows [16q,16q+16) read source rows [48-16q, 64-16q)
                src = slice(H - QH * (q + 1), H - QH * q)
                xq = xin_pool.tile([C, QH, W], f32, tag="xq", bufs=3)
                nc.sync.dma_start(xq[:, :, :], x[8 * L + 2][:, src, :])
                ybq = yb_pool.tile([C, QH, W], bf16, tag="ybq", bufs=3)
                nc.vector.scalar_tensor_tensor(
                    ybq[:, :, :], xq[:, ::-1, ::-1], 0.125,
                    accs[L][:, hq, :], mult, add,
                )
                nc.scalar.dma_start(y[L][:, hq, :], ybq[:, :, :])
    nc.compile()
    return nc


_NC_CACHE: list = []


def run(x: np.ndarray, trace: bool = False, **spmd_kwargs):
    """Shard, run on all 8 cores, gather. Returns (output, BassKernelResults)."""
    x = np.ascontiguousarray(x, dtype=np.float32)
    assert x.shape == (B, C, H, W), x.shape
    shards = x.reshape(N_CORES, ENTRIES_PER_CORE, C, H, W)
    if not _NC_CACHE:
        _NC_CACHE.append(build_nc())
    nc = _NC_CACHE[0]
    in_maps = [{"x": shards[i]} for i in range(N_CORES)]
    res = run_bass_kernel_spmd(
        nc, in_maps, list(range(N_CORES)), trace=trace, **spmd_kwargs
    )
    out = np.concatenate(
        [np.asarray(res.results[i]["y"]).astype(np.float32) for i in range(N_CORES)],
        axis=0,
    )
    return out, res


def kernel(x: np.ndarray) -> np.ndarray:
    out, _ = run(x)
    return out


# revision 8
# speedup vs baseline: 1.1216x; 1.1009x over previous
"""D4-pool Trainium2 kernel.

x: [256, 128, 64, 64] f32. Groups of 8 consecutive batch entries hold the 8
D4 orientations of one image; undo each orientation and mean over the group,
giving [32, 128, 64, 64].

Sharding: data-parallel over the group dim — core k gets groups [4k, 4k+4)
(batch entries [32k, 32k+32)), so the reduce is fully device-local.

Layout trick: with C (=128) on SBUF partitions and (H, W) on the free dim,
every D4 inverse transform is pure free-dim address arithmetic (stride ±1 /
±64 access patterns) — no transpose instructions, no partition movement.
Per partition, the required inverse-transform reads are:
  o=0: A[h, w]          o=1: A[w, 63-h]     o=2: A[63-h, 63-w]
  o=3: A[63-w, h]       o=4: A[h, 63-w]     o=5: A[w, h]
  o=6: A[63-h, w]       o=7: A[63-w, 63-h]

Two accumulators per group so only ONE DVE op per group pays the
slow inner-stride-64 (transposed) read:
  acc  [c,h,w]: init = x0*1/8 (ACT), += o=2,4,6 (flip APs, stride ±1)
  accT [c,w,h]: init = x5*1/8 (ACT; pure transpose == contiguous),
                += o=1,3,7 (flips in transposed coords, stride ±1)
The 1/8 scale folds into every accumulate (DVE STT: term*s + acc).

Accumulators are bf16 (inputs stream in as f32; each STT's multiply-add is
internally f32, only the running sum is rounded). Measured rel err ~5e-3 vs
the 2e-2 gate. This (a) halves accumulator SBUF so the load ring can run 9
tiles deep, (b) lets stores ship straight from the accumulator with no
staging copy, (c) makes the transposed combine an all-16-bit op.

Tail scheduling: the 16 DMA engines are the bottleneck (~27 GB/s x 16 =
~430 GB/s line rate); any compute serialized after the last load is pure
loss. One "early" group E has its first 7 orientations loaded and fully
reduced during the pipeline-fill phase (DVE is idle there), its accumulator
stays resident, and the kernel ends with E's o=2 tile streamed in four
quarter-tiles: load -> STT -> store at ~1.2 us granularity.
"""

import sys

for _p in ("/opt/trn_rl_repo",):
    if _p not in sys.path:
        sys.path.insert(0, _p)

import numpy as np

import concourse.bacc as bacc
import concourse.mybir as mybir
from concourse.bass_utils import run_bass_kernel_spmd
from concourse.tile import TileContext

N_CORES = 8
B, C, H, W = 256, 128, 64, 64
ENTRIES_PER_CORE = B // N_CORES          # 32 batch entries
GROUPS_PER_CORE = ENTRIES_PER_CORE // 8  # 4 groups of 8 orientations

# accT-side APs: accT[c,a,b] += x1[c,a,63-b] / x3[c,63-a,b] / x7[c,63-a,63-b]
_ACCT_AP = {
    1: lambda t: t[:, :, ::-1],
    3: lambda t: t[:, ::-1, :],
    7: lambda t: t[:, ::-1, ::-1],
}
# acc-side APs: acc[c,h,w] += x2[c,63-h,63-w] / x4[c,h,63-w] / x6[c,63-h,w]
_ACC_AP = {
    2: lambda t: t[:, ::-1, ::-1],
    4: lambda t: t[:, :, ::-1],
    6: lambda t: t[:, ::-1, :],
}


def build_nc(groups: int = GROUPS_PER_CORE) -> bacc.Bacc:
    f32 = mybir.dt.float32
    bf16 = mybir.dt.bfloat16
    mult, add = mybir.AluOpType.mult, mybir.AluOpType.add
    nc = bacc.Bacc()
    x = nc.declare_dram_parameter("x", [groups * 8, C, H, W], f32, isOutput=False)
    y = nc.declare_dram_parameter("y", [groups, C, H, W], bf16, isOutput=True)

    E = groups - 1           # the "early" group, finished last in quarters
    HH = H // 2              # half-tile rows
    QH = H // 4              # quarter-tile rows for the tail pipeline

    with TileContext(nc) as tc, nc.allow_low_precision("bf16 accumulate of 8-mean"):
        with (
            tc.tile_pool(name="xin", bufs=9) as xin_pool,
            tc.tile_pool(name="acc", bufs=2) as acc_pool,
            tc.tile_pool(name="accT", bufs=2) as accT_pool,
            tc.tile_pool(name="accE", bufs=1) as accE_pool,
        ):
            def load(b):
                xt = xin_pool.tile([C, H, W], f32, tag="xin")
                nc.sync.dma_start(xt[:, :, :], x[b])
                return xt

            def stt(dst, src_ap, hs=slice(None)):
                nc.vector.scalar_tensor_tensor(
                    dst[:, hs, :], src_ap, 0.125, dst[:, hs, :], mult, add,
                )

            def acct_phase(g, accT):
                """Loads + reduction of the accT side: o=5 (init), 1, 3, 7."""
                xt = load(8 * g + 5)
                nc.scalar.mul(accT[:, :, :], xt[:, :, :], 0.125)
                for o in (1, 3, 7):
                    xt = load(8 * g + o)
                    stt(accT, _ACCT_AP[o](xt))

            def combine(acc, accT):
                for h0 in (0, HH):
                    hs = slice(h0, h0 + HH)
                    nc.vector.tensor_add(
                        acc[:, hs, :], acc[:, hs, :],
                        accT[:, :, hs].transpose([0, 2, 1]),
                    )

            def acc_stt(acc, g, o):
                xt = load(8 * g + o)
                stt(acc, _ACC_AP[o](xt))

            # ---- early group E: everything except o=2, during pipeline fill
            accE = accE_pool.tile([C, H, W], bf16, tag="accE")
            xt = load(8 * E + 0)
            nc.scalar.mul(accE[:, :, :], xt[:, :, :], 0.125)
            accTE = accT_pool.tile([C, H, W], bf16, tag="accT")
            acct_phase(E, accTE)
            combine(accE, accTE)
            acc_stt(accE, E, 4)
            acc_stt(accE, E, 6)

            # ---- steady-state groups
            for g in range(groups - 1):
                acc = acc_pool.tile([C, H, W], bf16, tag="acc")
                xt = load(8 * g + 0)
                nc.scalar.mul(acc[:, :, :], xt[:, :, :], 0.125)
                accT = accT_pool.tile([C, H, W], bf16, tag="accT")
                acct_phase(g, accT)
                combine(acc, accT)
                acc_stt(acc, g, 4)
                acc_stt(acc, g, 6)
                xt2 = load(8 * g + 2)
                for h0 in (0, HH):
                    hs = slice(h0, h0 + HH)
                    stt(acc, _ACC_AP[2](xt2)[:, hs, :], hs)
                    # Store straight from the bf16 accumulator, on the ACT
                    # HWDGE queue so the compute-gated store doesn't
                    # head-of-line block loads on sync's queue.
                    nc.scalar.dma_start(y[g][:, hs, :], acc[:, hs, :])

            # ---- tail: E's o=2 in quarter-tiles, load->STT->store pipelined
            for q in range(4):
                hq = slice(QH * q, QH * (q + 1))
                # output rows [16q,16q+16) read source rows [48-16q, 64-16q)
                src = slice(H - QH * (q + 1), H - QH * q)
                xq = xin_pool.tile([C, QH, W], f32, tag="xq", bufs=3)
                nc.sync.dma_start(xq[:, :, :], x[8 * E + 2][:, src, :])
                stt(accE, xq[:, ::-1, ::-1], hq)
                nc.scalar.dma_start(y[E][:, hq, :], accE[:, hq, :])
    nc.compile()
    return nc


_NC_CACHE: list = []


def run(x: np.ndarray, trace: bool = False, **spmd_kwargs):
    """Shard, run on all 8 cores, gather. Returns (output, BassKernelResults)."""
    x = np.ascontiguousarray(x, dtype=np.float32)
    assert x.shape == (B, C, H, W), x.shape
    shards = x.reshape(N_CORES, ENTRIES_PER_CORE, C, H, W)
    if not _NC_CACHE:
        _NC_CACHE.append(build_nc())
    nc = _NC_CACHE[0]
    in_maps = [{"x": shards[i]} for i in range(N_CORES)]
    res = run_bass_kernel_spmd(
        nc, in_maps, list(range(N_CORES)), trace=trace, **spmd_kwargs
    )
    out = np.concatenate(
        [np.asarray(res.results[i]["y"]).astype(np.float32) for i in range(N_CORES)],
        axis=0,
    )
    return out, res


def kernel(x: np.ndarray) -> np.ndarray:
    out, _ = run(x)
    return out
